# revision 1
# baseline (speedup 1.0000x reference)
"""v2 of the 5x5 NN-loss kernel: paired shift groups, software-pipelined.

Same marshalled layout as the original (see marshal_core). The 25 shifts
are processed as 13 sub windows grouped into 8 units; units holding two
homogeneous pair-groups share one contiguous d tile so the adds and min
tree run at doubled width (the TensorTensor ISA caps APs at 3 free dims;
the shared-tile group dim merges uniformly with j, unlike the overlapping
sub windows, so only the downstream can widen). Per unit:

    group in0  = g tile AP [p, no(stride 780), 12(260), nk(stride 2), 256(1)]
    group in1  = pred AP   [p, no(stride 0),   12(256), nk(stride 0), 256(1)]
    d = in0 - in1          (one DVE tensor_tensor per group)
    |d| in place           (one ScalarE activation per group)
    s = d_c0 + d_c1; s += d_c2   (two DVE adds per group, in place)
    v = pairwise min, m = min(m, v)

Emission is software-pipelined with lookahead 2: each group's post-abs ops
are emitted after the sub of group k+2, so the in-order DVE never stalls
at an abs-dependent op while a ready sub sits behind it in the queue.

Defaults: g_odd is derived on-chip from g_even (SBUF->SBUF DMA column
shift + BIG pad memset on Pool) instead of loaded from HBM (-1.56MB of
3.87MB), and the head is split so the first j-half sub starts once pred
rows 0..6 and ge rows 0..6 land (pred on the Activation HWDGE queue,
g bands on SP).
"""

import numpy as np

NCORES = 8
BIG = np.float16(20000.0)

GE_COLS = 3 * 8 * 260
P_COLS = 3 * 4 * 256

_cache = {}

# (o_list, dj_list) per group; parity of dj_list[0] picks ge/go.
# Even-parity groups first (sorted by lowest ge row), odd-parity last.
GROUPS = (
    [((o,), (0, 2)) for o in (-2, -1, 0, 1, 2)]
    + [((-2, -1), (4,)), ((0, 1), (4,)), ((2,), (4,))]
    + [((o,), (1, 3)) for o in (-2, -1, 0, 1, 2)]
)

# quad variant: 4 quads (o-pair x dj-pair) + 4 pairs + 1 single
GROUPS_Q = [
    ((-2, -1), (0, 2)),
    ((0, 1), (0, 2)),
    ((2,), (0, 2)),
    ((-2, -1), (4,)),
    ((0, 1), (4,)),
    ((2,), (4,)),
    ((-2, -1), (1, 3)),
    ((0, 1), (1, 3)),
    ((2,), (1, 3)),
]


def _build_nc(
    repeat=1,
    bufs=2,
    dbufs=4,
    lookahead=2,
    macc_dma=0,
    dma_repeat=0,
    dma_split=0,
    quads=0,
    abs_split=0,
    body_off=0,
    loads_off=0,
    go_onchip=1,
    head_split=1,
    no_abs=0,
    dve_abs=0,
):
    from contextlib import ExitStack

    import concourse.bacc as bacc
    import concourse.mybir as mybir
    import concourse.tile as tile
    from concourse.ap import AP

    f16 = mybir.dt.float16
    f32 = mybir.dt.float32
    Alu = mybir.AluOpType

    nc = bacc.Bacc("TRN2", target_bir_lowering=False)
    ge_d = nc.dram_tensor("g_even", [128, GE_COLS], f16, kind="ExternalInput")
    go_d = nc.dram_tensor("g_odd", [128, GE_COLS], f16, kind="ExternalInput")
    p_d = nc.dram_tensor("pred", [128, P_COLS], f16, kind="ExternalInput")
    out_d = nc.dram_tensor("out", [128, 1], f32, kind="ExternalOutput")

    with ExitStack() as ctx:
        tc = ctx.enter_context(tile.TileContext(nc))
        pool = ctx.enter_context(tc.tile_pool(name="main", bufs=1))
        dpool = ctx.enter_context(tc.tile_pool(name="d", bufs=dbufs))
        spool = ctx.enter_context(tc.tile_pool(name="s", bufs=bufs))

        ge = pool.tile([128, GE_COLS], f16, tag="ge")
        go = pool.tile([128, GE_COLS], f16, tag="go")
        p = pool.tile([128, P_COLS], f16, tag="p")
        m = pool.tile([128, 1024], f16, tag="m")

        # pred on the Activation HWDGE queue, g tiles on SP: the two head
        # transfers run on different queues (parallel across DMA engines
        # on HW; the sim's shared-device model serializes them).
        band = 12 * 260
        if loads_off:
            # sliver loads: same deps/writers, ~zero bytes — isolates the
            # DMA-load cost differentially vs the full-load variant.
            nc.scalar.dma_start(out=p[:, 0:1], in_=p_d[:, 0:1])
            nc.sync.dma_start(out=ge[:, 0:1], in_=ge_d[:, 0:1])
            nc.sync.dma_start(out=go[:, 0:1], in_=go_d[:, 0:1])
        elif head_split:
            # finer head: the first j-half sub needs only pred rows 0..6
            # and ge rows 0..6 — land those first on both queues.
            nc.scalar.dma_start(out=p[:, 0:768], in_=p_d[:, 0:768])
            nc.sync.dma_start(out=ge[:, 0 : 3 * 260], in_=ge_d[:, 0 : 3 * 260])
            nc.scalar.dma_start(out=p[:, 768:1536], in_=p_d[:, 768:1536])
            nc.sync.dma_start(out=ge[:, 3 * 260 : 6 * 260], in_=ge_d[:, 3 * 260 : 6 * 260])
            nc.scalar.dma_start(out=p[:, 1536:P_COLS], in_=p_d[:, 1536:P_COLS])
            nc.sync.dma_start(out=ge[:, 6 * 260 : band], in_=ge_d[:, 6 * 260 : band])
            nc.sync.dma_start(out=ge[:, band:GE_COLS], in_=ge_d[:, band:GE_COLS])
            if not go_onchip:
                nc.sync.dma_start(out=go[:], in_=go_d[:])
        else:
            nc.scalar.dma_start(out=p[:], in_=p_d[:])
            nc.sync.dma_start(out=ge[:, 0:band], in_=ge_d[:, 0:band])
            nc.sync.dma_start(out=ge[:, band:GE_COLS], in_=ge_d[:, band:GE_COLS])
            if not go_onchip:
                nc.sync.dma_start(out=go[:], in_=go_d[:])

        if go_onchip and not loads_off:
            # go[r, 0:259] = ge[r, 1:260] via SBUF->SBUF DMA (no HBM
            # traffic, no compute-engine time); go[r, 259] is the constant
            # BIG pad column (memset on the idle Pool engine).
            src = AP(ge[:].tensor, 1, [[GE_COLS, 128], [260, 24], [1, 259]])
            dst = AP(go[:].tensor, 0, [[GE_COLS, 128], [260, 24], [1, 259]])
            nc.sync.dma_start(out=dst, in_=src)
            pad = AP(go[:].tensor, 259, [[GE_COLS, 128], [260, 24], [1, 1]])
            nc.gpsimd.memset(pad, float(BIG))

        if dma_repeat:
            # DMA throughput probe: extra full-size loads into scratch
            # tiles (no consumers). Differential slope over dma_repeat =
            # time per ~4MB load set on the real DMA path.
            dge = pool.tile([128, GE_COLS], f16, tag="dge")
            dgo = pool.tile([128, GE_COLS], f16, tag="dgo")
            dp = pool.tile([128, P_COLS], f16, tag="dp")
            for _ in range(dma_repeat):
                eng1 = nc.scalar if dma_split else nc.sync
                nc.sync.dma_start(out=dge[:], in_=ge_d[:])
                eng1.dma_start(out=dgo[:], in_=go_d[:])
                nc.sync.dma_start(out=dp[:], in_=p_d[:])

        ge_h = ge[:].tensor
        go_h = go[:].tensor
        p_h = p[:].tensor

        # units: pair-groups sharing one contiguous d tile; merged
        # downstream ops run at doubled width with <=3 free dims (the
        # group dim merges uniformly with j -- unlike the sub windows,
        # whose overlapping o dim cannot merge, hence subs stay per-pair).
        UNITS = [
            [((-2,), (0, 2)), ((-1,), (0, 2))],
            [((0,), (0, 2)), ((1,), (0, 2))],
            [((-2, -1), (4,)), ((0, 1), (4,))],
            [((2,), (0, 2)), ((2,), (1, 3))],
            [((-2,), (1, 3)), ((-1,), (1, 3))],
            [((0,), (1, 3)), ((1,), (1, 3))],
            [((2,), (4,))],
        ]
        units = UNITS * repeat
        K = len(units)
        state = {}
        first = [True]

        def emit_sub(k):
            unit = units[k]
            ntot = sum(len(o) * 12 * len(dj) * 256 for o, dj in unit)
            d = dpool.tile([128, ntot], f16, tag="d")
            off = 0
            for gi, (o_list, dj_list) in enumerate(unit):
                par = dj_list[0] % 2
                g_h = go_h if par else ge_h
                no, nk = len(o_list), len(dj_list)
                r0 = (o_list[0] + 2) * 3
                col = dj_list[0] - par
                n = no * 12 * nk * 256
                if head_split and k == 0 and gi == 0:
                    dview = d[:, off : off + n].rearrange(
                        "p (r k w) -> p r k w", r=12, k=nk, w=256
                    )
                    for rr, nr in ((0, 3), (3, 3), (6, 6)):
                        g_ap = AP(
                            g_h,
                            (r0 + rr) * 260 + col,
                            [[GE_COLS, 128], [260, nr], [2, nk], [1, 256]],
                        )
                        p_ap = AP(
                            p_h,
                            rr * 256,
                            [[P_COLS, 128], [256, nr], [0, nk], [1, 256]],
                        )
                        dslice = dview[:, rr : rr + nr]
                        nc.vector.tensor_tensor(dslice, g_ap, p_ap, Alu.subtract)
                        nc.scalar.activation(
                            dslice, dslice, mybir.ActivationFunctionType.Abs
                        )
                    off += n
                    continue
                if no == 1:
                    g_ap = AP(
                        g_h,
                        r0 * 260 + col,
                        [[GE_COLS, 128], [260, 12], [2, nk], [1, 256]],
                    )
                    p_ap = AP(
                        p_h, 0, [[P_COLS, 128], [256, 12], [0, nk], [1, 256]]
                    )
                    dview = d[:, off : off + n].rearrange(
                        "p (r k w) -> p r k w", r=12, k=nk, w=256
                    )
                else:
                    g_ap = AP(
                        g_h,
                        r0 * 260 + col,
                        [[GE_COLS, 128], [3 * 260, no], [260, 12], [1, 256]],
                    )
                    p_ap = AP(
                        p_h, 0, [[P_COLS, 128], [0, no], [256, 12], [1, 256]]
                    )
                    dview = d[:, off : off + n].rearrange(
                        "p (o r w) -> p o r w", o=no, r=12, w=256
                    )
                nc.vector.tensor_tensor(dview, g_ap, p_ap, Alu.subtract)
                if k == 0:
                    # head unit keeps per-member abs (finer early pipeline)
                    nc.scalar.activation(
                        d[:, off : off + n],
                        d[:, off : off + n],
                        mybir.ActivationFunctionType.Abs,
                    )
                off += n
            if k != 0:
                # the merged adds read the whole unit tile, so one
                # unit-wide abs loses no dependency granularity
                nc.scalar.activation(
                    d[:, 0:ntot], d[:, 0:ntot], mybir.ActivationFunctionType.Abs
                )
            state[k] = d

        def fold(cur, g, r):
            """min over the leading dim of cur viewed [p, g, r, 256]."""
            cv = cur[:].rearrange("p (g r w) -> p g r w", g=g, r=r, w=256)
            i0, i1 = cv[:, 0], cv[:, 1]
            if r == 4 and first[0]:
                ov = m[:].rearrange("p (r w) -> p r w", r=4, w=256)
                nc.vector.tensor_tensor(ov, i0, i1, Alu.min)
                first[0] = False
                return None
            nxt = spool.tile([128, r * 256], f16, tag=f"t{r}")
            ov = nxt[:].rearrange("p (r w) -> p r w", r=r, w=256)
            nc.vector.tensor_tensor(ov, i0, i1, Alu.min)
            return nxt

        def emit_down(k, last):
            unit = units[k]
            d = state.pop(k)
            ng = len(unit)
            no, nk = len(unit[0][0]), len(unit[0][1])
            ns = ng * no * 4 * nk * 256
            s = spool.tile([128, ns], f16, tag="s")
            x = ng * no * 4
            if nk == 2:
                dv = d[:].rearrange(
                    "p (x c k w) -> p x c k w", x=x, c=3, k=2, w=256
                )
                sv = s[:].rearrange("p (x k w) -> p x k w", x=x, k=2, w=256)
            else:
                dv = d[:].rearrange("p (x c w) -> p x c w", x=x, c=3, w=256)
                sv = s[:].rearrange("p (x w) -> p x w", x=x, w=256)
            nc.vector.tensor_tensor(sv, dv[:, :, 0], dv[:, :, 1], Alu.add)
            nc.vector.tensor_tensor(sv, sv, dv[:, :, 2], Alu.add)

            # min tree down to [p, 1024]
            cur = s
            if nk == 2:
                # fold k first: [p, x, k, w] -> [p, x, w]
                i0, i1 = sv[:, :, 0], sv[:, :, 1]
                if x == 4 and first[0]:
                    ov = m[:].rearrange("p (r w) -> p r w", r=4, w=256)
                    nc.vector.tensor_tensor(ov, i0, i1, Alu.min)
                    first[0] = False
                    cur = None
                else:
                    cur = spool.tile([128, x * 256], f16, tag="tk")
                    ov = cur[:].rearrange("p (r w) -> p r w", r=x, w=256)
                    nc.vector.tensor_tensor(ov, i0, i1, Alu.min)
            rows = x
            while cur is not None and rows > 4:
                cur = fold(cur, 2, rows // 2)
                rows //= 2
            if cur is None:
                return
            src = cur[:]
            if last:
                nc.vector.tensor_tensor(m[:], m[:], src, Alu.min)
                r1 = pool.tile([128, 1], f32, tag="r1")
                nc.vector.tensor_reduce(r1[:], m[:], mybir.AxisListType.X, Alu.add)
                nc.sync.dma_start(out=out_d[:], in_=r1[:])
            else:
                nc.vector.tensor_tensor(m[:], m[:], src, Alu.min)

        if body_off:
            nc.vector.memset(m[:], 0)
            r1 = pool.tile([128, 1], f32, tag="r1")
            nc.vector.tensor_reduce(r1[:], m[:], mybir.AxisListType.X, Alu.add)
            nc.sync.dma_start(out=out_d[:], in_=r1[:])
        else:
            for k in range(min(lookahead, K)):
                emit_sub(k)
            for k in range(K):
                if k + lookahead < K:
                    emit_sub(k + lookahead)
                emit_down(k, last=(k == K - 1))

    if not nc.is_finalized():
        nc.finalize()
    return nc


def marshal_core(pred2, gt2):
    """pred2, gt2: [2, 3, 256, 256] f32 -> core input dict (fp16 layouts)."""
    gtp = np.full((2, 3, 260, 262), BIG, np.float16)
    gtp[:, :, 2:258, 2:258] = gt2.astype(np.float16)
    sw = np.lib.stride_tricks.sliding_window_view(gtp, 8, axis=2)  # [2,3,253,262,8]
    sel = sw[:, :, 0:253:4]  # rows 4g -> [2,3,64,262,8] = (b,c,g,w,jj)
    base = sel.transpose(0, 2, 4, 1, 3)  # (b,g,jj,c,w) = [2,64,8,3,262]
    ge = np.ascontiguousarray(base[..., 0:260]).reshape(128, GE_COLS)
    go = np.ascontiguousarray(base[..., 1:261]).reshape(128, GE_COLS)
    p16 = (
        pred2.astype(np.float16)
        .reshape(2, 3, 64, 4, 256)  # (b,c,g,j,w)
        .transpose(0, 2, 3, 1, 4)  # (b,g,j,c,w)
        .reshape(128, P_COLS)
    )
    return {
        "g_even": ge,
        "g_odd": go,
        "pred": np.ascontiguousarray(p16),
    }


def kernel(pred_target, gt_target):
    from concourse.bass_utils import run_bass_kernel_spmd

    pred_target = np.asarray(pred_target)
    gt_target = np.asarray(gt_target)

    if "nc" not in _cache:
        _cache["nc"] = _build_nc()
    nc = _cache["nc"]

    in_maps = [
        marshal_core(pred_target[2 * ci : 2 * ci + 2], gt_target[2 * ci : 2 * ci + 2])
        for ci in range(NCORES)
    ]
    try:
        res = run_bass_kernel_spmd(nc, in_maps, core_ids=list(range(NCORES)))
    except ModuleNotFoundError:
        import os

        os.environ["BASS_NEVER_TRACE"] = "1"
        res = run_bass_kernel_spmd(nc, in_maps, core_ids=list(range(NCORES)))
    total = 0.0
    for r in res.results:
        total += float(r["out"].astype(np.float64).sum())
    return np.float32(total)

